# revision 21
# baseline (speedup 1.0000x reference)
"""Trainium2 Bass kernel for a dense transformer block (B=2, T=2048, C=1024,
H=16, Dff=4096), SPMD across 8 NeuronCores.

Sharding: attention is head-parallel (2 heads/core); one AllToAll per batch
redistributes attention output into a token-parallel layout; projection,
layernorms and the FFN run on each core's 512-token slice with full weights.

Schedule (v3): bf16 matmul paths; ACT does only exp+sqrt+LN-bias; diagonal
causal tiles compute exp only on the needed column suffix; batch-1 QKV is
emission-interleaved into batch-0's attention and proj-h0 into batch-1's
attention so the static per-engine order keeps the exp pipeline fed; QKV
chunk-pairs share LDWEIGHTS; FFN1 runs half-width only for the first 8 ff
tiles (to cover the second AllToAll) and paired-half full-width after; FFN2
streams LN2 stats so the tail is short.
"""

import sys
from itertools import chain

sys.path.insert(0, "/opt/trn_rl_repo")

import numpy as np
import ml_dtypes
import concourse.bacc as bacc
import concourse.mybir as mybir
import concourse.tile as tile
import concourse.bass_utils as bass_utils

try:  # make the NTFF profile shim importable as antenv.axon_hooks
    import antenv

    if "/opt/trn_rl_repo/antenv" not in antenv.__path__:
        antenv.__path__.append("/opt/trn_rl_repo/antenv")
except Exception:
    pass

f32 = mybir.dt.float32
f32r = mybir.dt.float32r
bf16 = mybir.dt.bfloat16
AF = mybir.ActivationFunctionType
ALU = mybir.AluOpType
BF = ml_dtypes.bfloat16

NC = 8          # cores
B = 2           # batch
T = 2048        # sequence length
C = 1024        # model dim
H = 16          # heads
HD = 64         # head dim
HPC = H // NC   # heads per core (2)
DH = HPC * HD   # per-core head cols (128)
DFF = 4096
TOK = B * T     # 4096 tokens
TOKC = TOK // NC  # 512 tokens per core
CT = C // 128   # 8 c-tiles
FT = DFF // 128  # 32 ff-tiles
KT = T // 128   # 16 k-tiles per batch
QC = T // 512   # 4 q-chunks of 512 per batch
HT = TOKC // B  # 256 tokens per batch per core
FSPL = 8        # ff tiles computed half-width (overlap cover)
LN_EPS = 1e-5

_CACHE = {}


def _build(dbg=False):
    nc = bacc.Bacc("TRN2", target_bir_lowering=False, debug=False, num_devices=NC)

    # ---- DRAM I/O (per-core values supplied via in_maps) ----
    xt_d = nc.dram_tensor("xt", [128, CT, TOK], bf16, kind="ExternalInput")
    wq_d = nc.dram_tensor("wq_c", [128, CT, 128], bf16, kind="ExternalInput")
    wk_d = nc.dram_tensor("wk_c", [128, CT, 128], bf16, kind="ExternalInput")
    wv_d = nc.dram_tensor("wv_c", [128, CT, 128], bf16, kind="ExternalInput")
    xres_d = nc.dram_tensor("xresb_c", [CT, 128, TOKC], bf16, kind="ExternalInput")
    wp_d = nc.dram_tensor("wproj", [CT, 128, C], bf16, kind="ExternalInput")
    w1_d = nc.dram_tensor("w1p", [FT, 128, CT, 128], bf16, kind="ExternalInput")
    w2_d = nc.dram_tensor("w2p", [CT, 128, FT, 128], bf16, kind="ExternalInput")
    bias_d = nc.dram_tensor("biaspack", [128, 5 * CT], f32, kind="ExternalInput")
    b1_d = nc.dram_tensor("b1t", [128, FT], f32, kind="ExternalInput")
    ones_d = nc.dram_tensor("onesp", [128, 128], f32r, kind="ExternalInput")
    ident_d = nc.dram_tensor("identb", [128, 128], bf16, kind="ExternalInput")
    mask_d = nc.dram_tensor("cmask2", [128, 4, HPC, 512], bf16, kind="ExternalInput")
    out_d = nc.dram_tensor("out", [C, TOKC], f32, kind="ExternalOutput")
    if dbg:
        dbg_att = nc.dram_tensor("dbg_att", [128, T], bf16,
                                 kind="ExternalOutput")
        dbg_x1 = nc.dram_tensor("dbg_x1", [CT, 128, TOKC], bf16,
                                kind="ExternalOutput")
        dbg_x2 = nc.dram_tensor("dbg_x2", [CT, 128, TOKC], bf16,
                                kind="ExternalOutput")

    with tile.TileContext(nc) as tc:
        with (
            nc.allow_low_precision(reason="bf16 matmul path (~0.4% rounding)"),
            tc.tile_pool(name="const", bufs=1) as p_const,
            tc.tile_pool(name="wqkv", bufs=1) as p_wqkv,
            tc.tile_pool(name="wp", bufs=CT) as p_wp,
            tc.tile_pool(name="agp", bufs=1) as p_ag,
            tc.tile_pool(name="x1p", bufs=CT) as p_x1,
            tc.tile_pool(name="ln1p", bufs=CT) as p_ln1,
            tc.tile_pool(name="xresp", bufs=CT) as p_xres,
            tc.tile_pool(name="dram", bufs=1, space="DRAM") as p_dram,
            tc.tile_pool(name="psmain", bufs=2, space="PSUM") as ps_main,
        ):
            # ---- startup-critical DMAs first: QKV weights + constants the
            # first attention steps need ----
            wq_sb = p_wqkv.tile([128, CT, 128], bf16, tag="wq")
            wk_sb = p_wqkv.tile([128, CT, 128], bf16, tag="wk")
            wv_sb = p_wqkv.tile([128, CT, 128], bf16, tag="wv")
            nc.sync.dma_start(wq_sb[:], wq_d[:])
            nc.sync.dma_start(wk_sb[:], wk_d[:])
            nc.sync.dma_start(wv_sb[:], wv_d[:])
            ones = p_const.tile([128, 128], f32r, tag="ones")
            nc.sync.dma_start(ones[:], ones_d[:])
            identb = p_const.tile([128, 128], bf16, tag="identb")
            nc.sync.dma_start(identb[:], ident_d[:])
            onesb = p_const.tile([128, 128], bf16, tag="onesb")
            nc.vector.tensor_copy(onesb[:], ones[:].bitcast(f32))
            masks = p_const.tile([128, 4, HPC, 512], bf16, tag="masks")
            nc.sync.dma_start(masks[:], mask_d[:])

            # created now, DMA-loaded after batch-0 attention is emitted (so
            # their transfers don't delay the first x chunks)
            biasp = p_const.tile([128, 5 * CT], f32, tag="biasp")
            b1t = p_const.tile([128, FT], f32, tag="b1t")
            wp_sb = [
                p_wp.tile([128, C], bf16, tag="wp", name=f"wp{i}")
                for i in range(CT)
            ]
            xres = [
                p_xres.tile([128, TOKC], bf16, tag="xres", name=f"xres{i}")
                for i in range(CT)
            ]
            # bias pack columns: [b2 | g1 | be1 | g2 | be2]
            b2_b = biasp[:, 0 * CT:1 * CT]
            g1_b = biasp[:, 1 * CT:2 * CT]
            be1_b = biasp[:, 2 * CT:3 * CT]
            g2_b = biasp[:, 3 * CT:4 * CT]
            be2_b = biasp[:, 4 * CT:5 * CT]

            def emit_deferred_loads():
                nc.sync.dma_start(biasp[:], bias_d[:])
                nc.sync.dma_start(b1t[:], b1_d[:])
                for kt in range(CT):
                    nc.sync.dma_start(wp_sb[kt][:], wp_d[kt])
                for ct in range(CT):
                    nc.sync.dma_start(xres[ct][:], xres_d[ct])

            a2a_in = [
                p_dram.tile([NC, DH, HT], bf16, tag=f"a2ai{b}", name=f"a2ai{b}")
                for b in range(B)
            ]
            a2a_out = [
                p_dram.tile([NC, DH, HT], bf16, tag=f"a2ao{b}", name=f"a2ao{b}")
                for b in range(B)
            ]
            ag = [
                [
                    p_ag.tile([128, HT], bf16, tag=f"ag{b}", bufs=NC,
                              name=f"ag{b}_{i}")
                    for i in range(NC)
                ]
                for b in range(B)
            ]
            x1 = [
                p_x1.tile([128, TOKC], bf16, tag="x1", name=f"x1_{i}")
                for i in range(CT)
            ]
            # LN1 output per token half (h0 reads never falsely order
            # against h1 writes)
            ln1h = [
                [
                    p_ln1.tile([128, HT], bf16, tag=f"ln1{hx}", bufs=CT,
                               name=f"ln1_{hx}_{i}")
                    for i in range(CT)
                ]
                for hx in range(2)
            ]

            halves = [slice(0, HT), slice(HT, TOKC)]

            def gen_proj(hx):
                """proj for token half hx: x1[:, half] = wp^T @ ag + xres."""
                hc = halves[hx]
                for mt in range(CT):
                    yps = ps_main.tile([128, HT], f32, tag="mm")
                    for kt in range(CT):
                        nc.tensor.matmul(
                            yps[:],
                            wp_sb[kt][:, mt * 128:(mt + 1) * 128],
                            ag[hx][kt][:],
                            start=(kt == 0), stop=(kt == CT - 1),
                        )
                        if kt % 2 == 1:
                            yield
                    # b_proj baked into xres on the host
                    nc.vector.tensor_add(x1[mt][:, hc], yps[:], xres[mt][:, hc])
                    yield

            def emit_ln(x_tiles, hc, g_b, be_b, out_fn, ps_pool, tmp_pool):
                """Feature-major LN on token columns `hc` of CT bf16 tiles."""
                nh = hc.stop - hc.start
                s1 = ps_pool.tile([1, nh], f32, tag="ln")
                s2 = ps_pool.tile([1, nh], f32, tag="ln")
                for ct in range(CT):
                    nc.tensor.matmul(
                        s1[:], onesb[:, 0:1], x_tiles[ct][:, hc],
                        start=(ct == 0), stop=(ct == CT - 1),
                    )
                for ct in range(CT):
                    sq = tmp_pool.tile([128, nh], bf16, tag=f"sq{nh}", bufs=2)
                    nc.vector.tensor_mul(
                        sq[:], x_tiles[ct][:, hc], x_tiles[ct][:, hc]
                    )
                    nc.tensor.matmul(
                        s2[:], onesb[:, 0:1], sq[:],
                        start=(ct == 0), stop=(ct == CT - 1),
                    )
                nmu = tmp_pool.tile([1, nh], f32r, tag=f"nmu{nh}", bufs=1)
                nc.vector.tensor_scalar_mul(nmu[:], s1[:], -1.0 / C)
                ex2 = tmp_pool.tile([1, nh], f32, tag=f"ex2{nh}", bufs=1)
                nc.vector.tensor_scalar_mul(ex2[:], s2[:], 1.0 / C)
                mu2 = tmp_pool.tile([1, nh], f32, tag=f"mu2{nh}", bufs=1)
                nc.vector.tensor_mul(
                    mu2[:], nmu[:].bitcast(f32), nmu[:].bitcast(f32)
                )
                var = tmp_pool.tile([1, nh], f32, tag=f"var{nh}", bufs=1)
                nc.vector.scalar_tensor_tensor(
                    var[:], ex2[:], LN_EPS, mu2[:], ALU.add, ALU.subtract
                )
                sd = tmp_pool.tile([1, nh], f32, tag=f"sd{nh}", bufs=1)
                nc.scalar.activation(sd[:], var[:], AF.Sqrt, bias=0.0)
                rsr = tmp_pool.tile([1, nh], f32, tag=f"rsr{nh}", bufs=1)
                nc.vector.reciprocal_approx_fast(rsr[:], sd[:])
                rsrr = tmp_pool.tile([1, nh], f32r, tag=f"rsrr{nh}", bufs=1)
                nc.vector.tensor_copy(rsrr[:], rsr[:])
                bmu = ps_pool.tile([128, nh], f32, tag="ln")
                nc.tensor.matmul(
                    bmu[:], ones[0:1, :], nmu[:], start=True, stop=True
                )
                brs = ps_pool.tile([128, nh], f32, tag="ln")
                nc.tensor.matmul(
                    brs[:], ones[0:1, :], rsrr[:], start=True, stop=True
                )
                for ct in range(CT):
                    t1 = tmp_pool.tile([128, nh], f32, tag=f"lt{nh}", bufs=2)
                    nc.vector.tensor_add(t1[:], x_tiles[ct][:, hc], bmu[:])
                    u = tmp_pool.tile([128, nh], f32, tag=f"lu{nh}", bufs=2)
                    nc.vector.scalar_tensor_tensor(
                        u[:], t1[:], g_b[:, ct:ct + 1], brs[:],
                        ALU.mult, ALU.mult,
                    )
                    # final bias add on ACT (idle outside attention)
                    nc.scalar.activation(
                        out_fn(ct), u[:], AF.Identity,
                        bias=be_b[:, ct:ct + 1],
                    )

            # ======== phase 1: QKV + attention (head-parallel) ========
            with (
                tc.tile_pool(name="attn", bufs=1) as p_attn,
                tc.tile_pool(name="xt", bufs=3) as p_xt,
                tc.tile_pool(name="qkv", bufs=1) as p_qkv,
                tc.tile_pool(name="es", bufs=1) as p_es,
                tc.tile_pool(name="small", bufs=2) as p_small,
                tc.tile_pool(name="psatt", bufs=1, space="PSUM") as ps_att,
            ):
                attnh = [
                    p_attn.tile([128, T], bf16, tag="attnh", bufs=2,
                                name=f"attnh{b}")
                    for b in range(B)
                ]
                qTc = [[None] * QC for _ in range(B)]
                kTc = [[None] * QC for _ in range(B)]
                vtc = [[None] * QC for _ in range(B)]

                def gen_qkv_pair(b, n0):
                    """QKV for chunks n0, n0+1 — shared LDWEIGHTS per ct."""
                    ns = (n0, n0 + 1)
                    xts = []
                    for n in ns:
                        xtc = p_xt.tile([128, CT, 512], bf16, tag="xt")
                        nc.sync.dma_start(
                            xtc[:],
                            xt_d[:, :, b * T + n * 512:b * T + (n + 1) * 512],
                        )
                        xts.append(xtc)
                    vcs = []
                    for w, kind in ((wq_sb, "q"), (wk_sb, "k"), (wv_sb, "v")):
                        pp = [
                            ps_main.tile([128, 512], f32, tag="mm",
                                         name=f"pp{kind}{i}")
                            for i in range(2)
                        ]
                        for ct in range(CT):
                            for i in range(2):
                                nc.tensor.matmul(
                                    pp[i][:], w[:, ct, :], xts[i][:, ct, :],
                                    start=(ct == 0), stop=(ct == CT - 1),
                                )
                            yield
                        for i, n in enumerate(ns):
                            if kind == "q":
                                dst = p_qkv.tile([128, 512], bf16, tag="qT",
                                                 bufs=8, name=f"qT{b}_{n}")
                                qTc[b][n] = dst
                            elif kind == "k":
                                dst = p_qkv.tile([128, 512], bf16, tag="kT",
                                                 bufs=8, name=f"kT{b}_{n}")
                                kTc[b][n] = dst
                            else:
                                dst = p_qkv.tile([128, 512], bf16, tag="vT",
                                                 bufs=2)
                                vcs.append(dst)
                            nc.vector.tensor_copy(dst[:], pp[i][:])
                        yield
                    for i, n in enumerate(ns):
                        vt = p_qkv.tile([128, 4, 132], bf16, tag="vt",
                                        bufs=8, name=f"vt{b}_{n}")
                        vtc[b][n] = vt
                        for ki in range(4):
                            pt = ps_main.tile([128, 128], bf16, tag="mm")
                            nc.tensor.transpose(
                                pt[:], vcs[i][:, ki * 128:(ki + 1) * 128],
                                identb[:],
                            )
                            nc.vector.tensor_copy(vt[:, ki, 0:64], pt[:, 0:64])
                            nc.vector.tensor_copy(vt[:, ki, 66:130],
                                                  pt[:, 64:128])
                            nc.vector.tensor_copy(vt[:, ki, 64:65],
                                                  onesb[:, 0:1])
                            nc.vector.tensor_copy(vt[:, ki, 130:131],
                                                  onesb[:, 1:2])
                            yield

                def emit_attention(b, filler=None):
                    def step():
                        if filler is not None:
                            next(filler, None)

                    for j in range(QC):
                        nkt = 4 * j + 4
                        oacc = [
                            ps_att.tile([65, 512], f32, tag="oacc", bufs=2,
                                        name=f"oacc{b}_{j}_{h}")
                            for h in range(HPC)
                        ]
                        for kt in range(nkt):
                            m = kt - 4 * j
                            c0 = 0 if m < 0 else 128 * m
                            spair = ps_att.tile([128, HPC, 512], f32,
                                                tag="sp", bufs=2)
                            for h in range(HPC):
                                hrow = slice(h * 64, (h + 1) * 64)
                                nc.tensor.matmul(
                                    spair[:, h, c0:512],
                                    kTc[b][kt // 4][hrow,
                                                    (kt % 4) * 128:
                                                    (kt % 4 + 1) * 128],
                                    qTc[b][j][hrow, c0:512],
                                    start=True, stop=True,
                                    tile_position=(64 * h, 0),
                                )
                            epair = p_es.tile([128, HPC, 512], bf16, tag="es",
                                              bufs=6)
                            if m < 0:
                                nc.scalar.activation(
                                    epair[:], spair[:], AF.Exp, scale=0.125
                                )
                            else:
                                ed = p_es.tile([128, HPC, 512], bf16,
                                               tag="ed", bufs=2)
                                nc.scalar.activation(
                                    ed[:, :, c0:512], spair[:, :, c0:512],
                                    AF.Exp, scale=0.125,
                                )
                                nc.vector.tensor_mul(
                                    epair[:, :, c0:512], ed[:, :, c0:512],
                                    masks[:, m, :, c0:512],
                                )
                            vt = vtc[b][kt // 4]
                            for h in range(HPC):
                                nc.tensor.matmul(
                                    oacc[h][:, c0:512],
                                    vt[:, kt % 4, 66 * h:66 * h + 65],
                                    epair[:, h, c0:512],
                                    start=(kt == 0), stop=(kt == nkt - 1),
                                )
                            step()
                        for h in range(HPC):
                            osr = p_small.tile([64, 512], f32, tag="osr",
                                               bufs=2)
                            nc.vector.tensor_copy(osr[:], oacc[h][0:64, :])
                            rrow = p_small.tile([1, 512], f32r, tag="rrow",
                                                bufs=2)
                            nc.vector.tensor_copy(rrow[:], oacc[h][64:65, :])
                            bps = ps_att.tile([64, 512], f32, tag="oacc",
                                              bufs=2)
                            nc.tensor.matmul(
                                bps[:], ones[0:1, 0:64], rrow[:],
                                start=True, stop=True,
                            )
                            ibc = p_small.tile([64, 512], f32, tag="ibc",
                                               bufs=2)
                            nc.vector.reciprocal_approx_fast(ibc[:], bps[:])
                            nc.vector.tensor_mul(
                                attnh[b][64 * h:64 * h + 64,
                                         j * 512:(j + 1) * 512],
                                osr[:],
                                ibc[:],
                            )
                            step()
                        for s in (2 * j, 2 * j + 1):
                            nc.sync.dma_start(
                                a2a_in[b][s],
                                attnh[b][:, s * HT:(s + 1) * HT],
                            )

                # -------- batch 0 QKV (dense; nothing else to overlap) ----
                for _ in gen_qkv_pair(0, 0):
                    pass
                for _ in gen_qkv_pair(0, 2):
                    pass
                # batch-0 attention with batch-1 QKV as PE filler
                b1fill = chain(gen_qkv_pair(1, 0), gen_qkv_pair(1, 2))
                emit_attention(0, filler=b1fill)
                for _ in b1fill:
                    pass
                if dbg:
                    nc.sync.dma_start(dbg_att[:], attnh[0][:])
                nc.gpsimd.collective_compute(
                    "AllToAll",
                    ALU.bypass,
                    replica_groups=[list(range(NC))],
                    ins=[a2a_in[0][:].opt()],
                    outs=[a2a_out[0][:].opt()],
                )
                emit_deferred_loads()
                for i in range(NC):
                    nc.sync.dma_start(ag[0][i][:], a2a_out[0][i])

                # batch-1 attention with proj-h0 as PE filler
                p0fill = gen_proj(0)
                emit_attention(1, filler=p0fill)
                for _ in p0fill:
                    pass
                nc.gpsimd.collective_compute(
                    "AllToAll",
                    ALU.bypass,
                    replica_groups=[list(range(NC))],
                    ins=[a2a_in[1][:].opt()],
                    outs=[a2a_out[1][:].opt()],
                )
                for i in range(NC):
                    nc.sync.dma_start(ag[1][i][:], a2a_out[1][i])

            # ======== phase 2: LN1 + FFN + LN2 (token-parallel) ========
            with (
                tc.tile_pool(name="hff", bufs=FT) as p_hff,
                tc.tile_pool(name="w1s", bufs=4) as p_w1,
                tc.tile_pool(name="w2s", bufs=2) as p_w2,
                tc.tile_pool(name="x2p", bufs=CT) as p_x2,
                tc.tile_pool(name="tmp3", bufs=3) as p_tmp3,
                tc.tile_pool(name="outp", bufs=2) as p_out,
                tc.tile_pool(name="psln", bufs=2, space="PSUM") as ps_ln,
            ):
                hff = [
                    p_hff.tile([128, TOKC], bf16, tag="hff", name=f"hff{i}")
                    for i in range(FT)
                ]
                x2 = [
                    p_x2.tile([128, TOKC], bf16, tag="x2", name=f"x2_{i}")
                    for i in range(CT)
                ]

                def emit_ffn1_half(hx, ftr):
                    hc = halves[hx]
                    for ft in ftr:
                        w1t = p_w1.tile([128, CT, 128], bf16, tag="w1")
                        nc.sync.dma_start(w1t[:], w1_d[ft])
                        yps = ps_main.tile([128, HT], f32, tag="mm")
                        for kt in range(CT):
                            nc.tensor.matmul(
                                yps[:], w1t[:, kt, :], ln1h[hx][kt][:],
                                start=(kt == 0), stop=(kt == CT - 1),
                            )
                        nc.vector.tensor_scalar(
                            hff[ft][:, hc], yps[:], b1t[:, ft:ft + 1], 0.0,
                            ALU.add, ALU.max,
                        )

                def emit_ffn1_pair(ftr):
                    """Full-width FFN1: both halves share each LDWEIGHTS."""
                    for ft in ftr:
                        w1t = p_w1.tile([128, CT, 128], bf16, tag="w1")
                        nc.sync.dma_start(w1t[:], w1_d[ft])
                        yps = ps_main.tile([128, TOKC], f32, tag="mm")
                        for kt in range(CT):
                            for hx in range(2):
                                nc.tensor.matmul(
                                    yps[:, halves[hx]],
                                    w1t[:, kt, :], ln1h[hx][kt][:],
                                    start=(kt == 0 and hx == 0),
                                    stop=(kt == CT - 1 and hx == 1),
                                )
                        nc.vector.tensor_scalar(
                            hff[ft][:], yps[:], b1t[:, ft:ft + 1], 0.0,
                            ALU.add, ALU.max,
                        )

                # half 0: LN1 + first ff tiles (covers the second AllToAll)
                emit_ln(
                    x1, halves[0], g1_b, be1_b,
                    lambda ct: ln1h[0][ct][:], ps_ln, p_tmp3,
                )
                emit_ffn1_half(0, range(FSPL))

                # half 1, then the remaining ff tiles full-width
                for _ in gen_proj(1):
                    pass
                if dbg:
                    for ct in range(CT):
                        nc.sync.dma_start(dbg_x1[ct], x1[ct][:])
                emit_ln(
                    x1, halves[1], g1_b, be1_b,
                    lambda ct: ln1h[1][ct][:], ps_ln, p_tmp3,
                )
                emit_ffn1_half(1, range(FSPL))
                emit_ffn1_pair(range(FSPL, FT))

                # ---- FFN2 (mt-outer, streaming LN2 stats) ----
                s1 = ps_ln.tile([1, TOKC], f32, tag="ln")
                s2 = ps_ln.tile([1, TOKC], f32, tag="ln")
                for mt in range(CT):
                    w2t = p_w2.tile([128, FT, 128], bf16, tag="w2")
                    nc.sync.dma_start(w2t[:], w2_d[mt])
                    acc = ps_main.tile([128, TOKC], f32, tag="mm")
                    for ft in range(FT):
                        nc.tensor.matmul(
                            acc[:], w2t[:, ft, :], hff[ft][:],
                            start=(ft == 0), stop=(ft == FT - 1),
                        )
                    for hx in range(2):
                        hc = halves[hx]
                        nc.vector.scalar_tensor_tensor(
                            x2[mt][:, hc], acc[:, hc], b2_b[:, mt:mt + 1],
                            ln1h[hx][mt][:], ALU.add, ALU.add,
                        )
                    sq = p_tmp3.tile([128, TOKC], bf16, tag="sq2", bufs=2)
                    nc.vector.tensor_mul(sq[:], x2[mt][:], x2[mt][:])
                    nc.tensor.matmul(
                        s1[:], onesb[:, 0:1], x2[mt][:],
                        start=(mt == 0), stop=(mt == CT - 1),
                    )
                    nc.tensor.matmul(
                        s2[:], onesb[:, 0:1], sq[:],
                        start=(mt == 0), stop=(mt == CT - 1),
                    )
                if dbg:
                    for ct in range(CT):
                        nc.sync.dma_start(dbg_x2[ct], x2[ct][:])

                # ---- LN2 finalize (stats already accumulated) ----
                nh = TOKC
                nmu = p_tmp3.tile([1, nh], f32r, tag="nmu2", bufs=1)
                nc.vector.tensor_scalar_mul(nmu[:], s1[:], -1.0 / C)
                ex2 = p_tmp3.tile([1, nh], f32, tag="ex22", bufs=1)
                nc.vector.tensor_scalar_mul(ex2[:], s2[:], 1.0 / C)
                mu2 = p_tmp3.tile([1, nh], f32, tag="mu22", bufs=1)
                nc.vector.tensor_mul(
                    mu2[:], nmu[:].bitcast(f32), nmu[:].bitcast(f32)
                )
                var = p_tmp3.tile([1, nh], f32, tag="var2", bufs=1)
                nc.vector.scalar_tensor_tensor(
                    var[:], ex2[:], LN_EPS, mu2[:], ALU.add, ALU.subtract
                )
                sd = p_tmp3.tile([1, nh], f32, tag="sd2", bufs=1)
                nc.scalar.activation(sd[:], var[:], AF.Sqrt, bias=0.0)
                rsr = p_tmp3.tile([1, nh], f32, tag="rsr2", bufs=1)
                nc.vector.reciprocal_approx_fast(rsr[:], sd[:])
                rsrr = p_tmp3.tile([1, nh], f32r, tag="rsrr2", bufs=1)
                nc.vector.tensor_copy(rsrr[:], rsr[:])
                bmu = ps_ln.tile([128, nh], f32, tag="ln")
                nc.tensor.matmul(
                    bmu[:], ones[0:1, :], nmu[:], start=True, stop=True
                )
                brs = ps_ln.tile([128, nh], f32, tag="ln")
                nc.tensor.matmul(
                    brs[:], ones[0:1, :], rsrr[:], start=True, stop=True
                )
                for ct in range(CT):
                    t1 = p_tmp3.tile([128, nh], f32, tag="lt2", bufs=2)
                    nc.vector.tensor_add(t1[:], x2[ct][:], bmu[:])
                    u = p_tmp3.tile([128, nh], f32, tag="lu2", bufs=2)
                    nc.vector.scalar_tensor_tensor(
                        u[:], t1[:], g2_b[:, ct:ct + 1], brs[:],
                        ALU.mult, ALU.mult,
                    )
                    ot = p_out.tile([128, nh], f32, tag="outt")
                    nc.scalar.activation(
                        ot[:], u[:], AF.Identity, bias=be2_b[:, ct:ct + 1]
                    )
                    nc.sync.dma_start(
                        out_d[ct * 128:(ct + 1) * 128, :], ot[:]
                    )

    nc.compile()
    return nc


def _pack_inputs(inputs):
    """Host-side sharding/marshalling. Returns in_maps for the 8 cores."""
    x = np.asarray(inputs["x"], dtype=np.float32)
    xf = x.reshape(TOK, C)
    xt = np.ascontiguousarray(xf.T)  # [C, TOK]
    xt_p = np.ascontiguousarray(
        xt.reshape(CT, 128, TOK).transpose(1, 0, 2)
    ).astype(BF)  # [128, CT, TOK]
    wq = np.asarray(inputs["wq"], dtype=np.float32)
    wk = np.asarray(inputs["wk"], dtype=np.float32)
    wv = np.asarray(inputs["wv"], dtype=np.float32)
    bproj = np.asarray(inputs["b_proj"], dtype=np.float32)
    wp_p = np.ascontiguousarray(
        np.asarray(inputs["w_proj"], dtype=np.float32).reshape(CT, 128, C)
    ).astype(BF)
    w1 = np.asarray(inputs["w1"], dtype=np.float32)
    w2 = np.asarray(inputs["w2"], dtype=np.float32)
    # w1 packed per ff-tile: [FT, 128(c within tile), CT, 128(ff)]
    w1p = np.ascontiguousarray(
        w1.reshape(CT, 128, FT, 128).transpose(2, 1, 0, 3)
    ).astype(BF)
    # w2 packed per c-tile (mt): [CT, 128(ff within tile), FT, 128(c)]
    w2p = np.ascontiguousarray(
        w2.reshape(FT, 128, CT, 128).transpose(2, 1, 0, 3)
    ).astype(BF)

    def tile_vec(v, n):
        return np.ascontiguousarray(
            np.asarray(v, dtype=np.float32).reshape(n, 128).T
        )

    biaspack = np.zeros((128, 5 * CT), dtype=np.float32)
    biaspack[:, 0 * CT:1 * CT] = tile_vec(inputs["b2"], CT)
    biaspack[:, 1 * CT:2 * CT] = tile_vec(inputs["g1"], CT)
    biaspack[:, 2 * CT:3 * CT] = tile_vec(inputs["be1"], CT)
    biaspack[:, 3 * CT:4 * CT] = tile_vec(inputs["g2"], CT)
    biaspack[:, 4 * CT:5 * CT] = tile_vec(inputs["be2"], CT)
    b1t = tile_vec(inputs["b1"], FT)

    # causal masks for the 4 diagonal offsets, duplicated per head:
    # [128, 4, HPC, 512]
    r = np.arange(128)[:, None]
    ccol = np.arange(512)[None, :]
    cm = np.stack(
        [(ccol >= r + 128 * m).astype(np.float32) for m in range(4)], axis=1
    )  # [128, 4, 512]
    cmask2 = np.ascontiguousarray(
        np.repeat(cm[:, :, None, :], HPC, axis=2)
    ).astype(BF)
    onesp = np.ones((128, 128), dtype=np.float32)
    identb = np.eye(128, dtype=np.float32).astype(BF)

    in_maps = []
    for c in range(NC):
        hcol = slice(c * DH, (c + 1) * DH)

        def pack_w(w):
            return np.ascontiguousarray(
                w[:, hcol].reshape(CT, 128, DH).transpose(1, 0, 2)
            ).astype(BF)

        xres_c = np.concatenate(
            [
                xt[:, b * T + c * HT:b * T + (c + 1) * HT]
                for b in range(B)
            ],
            axis=1,
        ) + bproj[:, None]
        xresb_c = np.ascontiguousarray(
            xres_c.reshape(CT, 128, TOKC)
        ).astype(BF)

        in_maps.append(
            {
                "xt": xt_p,
                "wq_c": pack_w(wq),
                "wk_c": pack_w(wk),
                "wv_c": pack_w(wv),
                "xresb_c": xresb_c,
                "wproj": wp_p,
                "w1p": w1p,
                "w2p": w2p,
                "biaspack": biaspack,
                "b1t": b1t,
                "onesp": onesp,
                "identb": identb,
                "cmask2": cmask2,
            }
        )
    return in_maps


def _run(inputs, trace=False, debug=False):
    key = "dbg" if debug else "rel"
    if key not in _CACHE:
        _CACHE[key] = _build(dbg=debug)
    nc = _CACHE[key]
    in_maps = _pack_inputs(inputs)
    res = bass_utils.run_bass_kernel_spmd(
        nc, in_maps, core_ids=list(range(NC)), trace=trace
    )
    out = np.empty((TOK, C), dtype=np.float32)
    for c in range(NC):
        oc = res.results[c]["out"]
        for b in range(B):
            out[b * T + c * HT:b * T + (c + 1) * HT, :] = (
                oc[:, b * HT:(b + 1) * HT].T
            )
    return out.reshape(B, T, C), res


def kernel(**inputs) -> np.ndarray:
    out, _ = _run(inputs, trace=False, debug=False)
    return out
